# revision 9
# baseline (speedup 1.0000x reference)
"""Trainium2 Bass kernel for fused ConvTranspose2d -> *0.5 -> global spatial mean.

Problem (hardcoded shapes):
  x      [64, 64, 128, 128] f32
  weight [64, 64, 3, 3]     f32  (ConvTranspose2d layout [Cin, Cout, kH, kW])
  bias   [64]               f32
  out    [64, 64, 1, 1]     f32
  stride=2, pad=1, output_padding=1 -> Hout=Wout=256.

Math: the conv-transpose + global mean collapses algebraically. With the
validity masks Vh/Vw (all ones except [0,0]), the masked spatial sums per
kernel offset are expressible from four scalars per (b, c):
  T   = sum of the whole 128x128 image
  R0  = sum of row h=0
  C0  = sum of col w=0
  X   = x[b, c, 0, 0]
  acc[b,o] = sum_c  T*Wsum[c,o] - R0*Wrow0[c,o] - C0*Wcol0[c,o] + X*W00[c,o]
  out[b,o] = (acc / 65536 + bias[o]) * 0.5
where Wsum = sum over 3x3, Wrow0 = sum of kh=0 row, Wcol0 = sum of kw=0 col,
W00 = weight[c,o,0,0].

Sharding: data-parallel over batch across 8 cores (8 batches/core, 32 MiB of
x per core); weight/bias replicated; per-core output [8, 64]; host concat.

Per-core kernel: stream x as 4 tiles of [128, 16384] f32 (partition =
(b2, c) over 2 batches, free = h*w), reduce on the vector engine, then a few
tiny PE matmuls contract stats [128, 8] against weight features [128, 64]
(channel features duplicated across both batch halves; batch separation via
zero-padded stat columns).
"""

import os
import sys

import numpy as np

_TRN_REPO = "/opt/trn_rl_repo"
if _TRN_REPO not in sys.path and os.path.isdir(_TRN_REPO):
    sys.path.insert(0, _TRN_REPO)

import concourse.bass as bass
import concourse.tile as tile
from concourse import bacc, mybir
from concourse.bass_utils import run_bass_kernel_spmd

B, CIN, HIN, WIN = 64, 64, 128, 128
COUT, K = 64, 3
N_CORES = 8
B_PER_CORE = B // N_CORES          # 8
HW = HIN * WIN                     # 16384
N_DBL = B_PER_CORE // 2            # 4 double-batch tiles per core
FP32 = mybir.dt.float32
AX = mybir.AxisListType
ADD = mybir.AluOpType.add

_CACHE = {}


def _build_bass():
    nc = bacc.Bacc("TRN2", target_bir_lowering=False, debug=False, num_devices=N_CORES)

    x_d = nc.dram_tensor("x", [B_PER_CORE, CIN, HIN, WIN], FP32, kind="ExternalInput")
    w_d = nc.dram_tensor("weight", [CIN, COUT, K, K], FP32, kind="ExternalInput")
    b_d = nc.dram_tensor("bias", [COUT], FP32, kind="ExternalInput")
    o_d = nc.dram_tensor("out", [B_PER_CORE, COUT], FP32, kind="ExternalOutput")

    with tile.TileContext(nc) as tc:
        with (
            tc.tile_pool(name="const", bufs=1) as const,
            tc.tile_pool(name="xin", bufs=8) as xin,
            tc.tile_pool(name="tmp", bufs=8) as tmp,
            tc.tile_pool(name="psum", bufs=1, space="PSUM") as psum,
        ):
            # ---- weight / bias prep (tiny) ----
            w_sb = const.tile([CIN, COUT * K * K], FP32, tag="w_sb")
            nc.gpsimd.dma_start(w_sb[:], w_d[:].rearrange("c o kh kw -> c (o kh kw)"))
            bias_sb = const.tile([1, COUT], FP32, tag="bias_sb")
            nc.gpsimd.dma_start(bias_sb[:], b_d[:].rearrange("(a o) -> a o", a=1))

            w3 = w_sb[:].rearrange("c (o e) -> c o e", e=K * K)      # [64,64,9]
            w4 = w_sb[:].rearrange("c (o kh kw) -> c o kh kw", kh=K, kw=K)

            wsum = const.tile([CIN, COUT], FP32, tag="wsum")
            nc.vector.tensor_reduce(wsum[:], w3, axis=AX.X, op=ADD)
            wrow0 = const.tile([CIN, COUT], FP32, tag="wrow0")
            nc.vector.tensor_reduce(wrow0[:], w4[:, :, 0:1, :], axis=AX.XY, op=ADD)
            wcol0 = const.tile([CIN, COUT], FP32, tag="wcol0")
            nc.vector.tensor_reduce(wcol0[:], w4[:, :, :, 0:1], axis=AX.XY, op=ADD)

            # Weight-feature matrices [128, 64]: channel feature duplicated
            # across both batch halves (partition p = b2*64 + c); sign folded.
            wf_s = const.tile([128, COUT], FP32, tag="wf_s")
            wf_r = const.tile([128, COUT], FP32, tag="wf_r")
            wf_c = const.tile([128, COUT], FP32, tag="wf_c")
            wf_x = const.tile([128, COUT], FP32, tag="wf_x")
            for half in range(2):
                dst = slice(half * 64, half * 64 + 64)
                nc.scalar.mul(wf_s[dst, :], wsum[:], 1.0)
                nc.scalar.mul(wf_r[dst, :], wrow0[:], -1.0)
                nc.scalar.mul(wf_c[dst, :], wcol0[:], -1.0)
                nc.scalar.mul(wf_x[dst, :], w3[:, :, 0:1], 1.0)

            # ---- stat tensors [128, 8] (zero-padded; col = global batch) ----
            st_s = const.tile([128, B_PER_CORE], FP32, tag="st_s")
            st_r = const.tile([128, B_PER_CORE], FP32, tag="st_r")
            st_c = const.tile([128, B_PER_CORE], FP32, tag="st_c")
            st_x = const.tile([128, B_PER_CORE], FP32, tag="st_x")
            for st in (st_s, st_r, st_c, st_x):
                nc.vector.memset(st[:], 0.0)

            # ---- main loop: stream x, reduce ----
            # Chunk-wise partial reductions: each reduce waits on exactly one
            # DMA (walrus caps sync-wait commands per instruction) and starts
            # as soon as its chunk lands.
            N_CHUNK = 4
            CHUNK = HW // N_CHUNK            # 4096 elems = 32 rows of 128
            ROWS_PER_CHUNK = CHUNK // WIN
            for d in range(N_DBL):
                src = x_d[2 * d : 2 * d + 2].rearrange("b c h w -> (b c) (h w)")
                part_s = tmp.tile([128, N_CHUNK], FP32, tag="part_s")
                part_c = tmp.tile([128, N_CHUNK], FP32, tag="part_c")
                tr = tmp.tile([128, 1], FP32, tag="tr")
                for j in range(N_CHUNK):
                    ct = xin.tile([128, CHUNK], FP32, tag="ct")
                    sl = slice(j * CHUNK, (j + 1) * CHUNK)
                    nc.gpsimd.dma_start(ct[:], src[:, sl])
                    # partial full-image sum for this chunk
                    nc.vector.tensor_reduce(
                        part_s[:, j : j + 1], ct[:], axis=AX.X, op=ADD
                    )
                    # partial col-0 sum (stride-128 within the chunk)
                    ch = ct[:].rearrange("p (h w) -> p h w", w=WIN)
                    nc.vector.tensor_reduce(
                        part_c[:, j : j + 1], ch[:, :, 0:1], axis=AX.XY, op=ADD
                    )
                    if j == 0:
                        # row-0 sum (first 128 elements) and corner pixel
                        nc.vector.tensor_reduce(
                            tr[:], ct[:, 0:WIN], axis=AX.X, op=ADD
                        )
                        for b2 in range(2):
                            p = slice(b2 * 64, b2 * 64 + 64)
                            col = slice(2 * d + b2, 2 * d + b2 + 1)
                            nc.vector.tensor_copy(st_x[p, col], ct[p, 0:1])

                # combine partials
                ts = tmp.tile([128, 1], FP32, tag="ts")
                nc.vector.tensor_reduce(ts[:], part_s[:], axis=AX.X, op=ADD)
                tcl = tmp.tile([128, 1], FP32, tag="tcl")
                nc.vector.tensor_reduce(tcl[:], part_c[:], axis=AX.X, op=ADD)

                # scatter halves into the right (partition-range, column) slot
                for b2 in range(2):
                    p = slice(b2 * 64, b2 * 64 + 64)
                    col = slice(2 * d + b2, 2 * d + b2 + 1)
                    nc.scalar.mul(st_s[p, col], ts[p, :], 1.0)
                    nc.scalar.mul(st_r[p, col], tr[p, :], 1.0)
                    nc.scalar.mul(st_c[p, col], tcl[p, :], 1.0)

            # ---- contraction on PE ----
            acc = psum.tile([B_PER_CORE, COUT], FP32, tag="acc")
            nc.tensor.matmul(acc[:], st_s[:], wf_s[:], start=True, stop=False)
            nc.tensor.matmul(acc[:], st_r[:], wf_r[:], start=False, stop=False)
            nc.tensor.matmul(acc[:], st_c[:], wf_c[:], start=False, stop=False)
            nc.tensor.matmul(acc[:], st_x[:], wf_x[:], start=False, stop=False)
            # bias via rank-1 update: ones[1,8] (=65536) x bias[1,64]
            ones = const.tile([1, B_PER_CORE], FP32, tag="ones")
            nc.vector.memset(ones[:], float(HW * 4))  # Hout*Wout = 65536
            nc.tensor.matmul(acc[:], ones[:], bias_sb[:], start=False, stop=True)

            out_sb = const.tile([B_PER_CORE, COUT], FP32, tag="out_sb")
            nc.scalar.mul(out_sb[:], acc[:], 0.5 / float(HW * 4))
            nc.gpsimd.dma_start(o_d[:], out_sb[:])

    nc.compile()
    return nc


def _get_nc():
    if "nc" not in _CACHE:
        _CACHE["nc"] = _build_bass()
    return _CACHE["nc"]


def kernel(x, weight, bias, _trace=False, _tmpdir=None):
    x = np.ascontiguousarray(np.asarray(x, dtype=np.float32))
    weight = np.ascontiguousarray(np.asarray(weight, dtype=np.float32))
    bias = np.ascontiguousarray(np.asarray(bias, dtype=np.float32))
    assert x.shape == (B, CIN, HIN, WIN), x.shape

    nc = _get_nc()
    in_maps = [
        {
            "x": x[i * B_PER_CORE : (i + 1) * B_PER_CORE],
            "weight": weight,
            "bias": bias,
        }
        for i in range(N_CORES)
    ]
    res = run_bass_kernel_spmd(
        nc, in_maps, list(range(N_CORES)), trace=_trace, tmpdir=_tmpdir
    )
    _CACHE["last_results"] = res
    out = np.concatenate([res.results[i]["out"] for i in range(N_CORES)], axis=0)
    return out.reshape(B, COUT, 1, 1).astype(np.float32)
